# revision 1
# baseline (speedup 1.0000x reference)
"""Trainium2 Bass kernel for nn_BatchedTeacherPolicy.

2048 independent per-teacher MLPs (obs-norm -> 48->512->256->128->12,
ELU between layers, tanh at the end). Pure data parallel: 256 teachers
per NeuronCore across 8 cores.

Layout: teacher-on-partition. Each SBUF partition holds one teacher's
weights/activations; the per-teacher matvec y[o] = b[o] + sum_i W[o,i]x[i]
is one fused DVE tensor_tensor_reduce per output neuron o, computed for
128 teachers (partitions) simultaneously. Weight DMAs are fully
contiguous per partition (W[n, o0:o1, :] blocks).
"""

from contextlib import ExitStack

import numpy as np

import concourse.bass as bass
import concourse.bacc as bacc
import concourse.tile as tile
from concourse import mybir
from concourse.bass_utils import run_bass_kernel_spmd

N, OBS = 2048, 48
DIMS = [(512, 48), (256, 512), (128, 256), (12, 128)]  # (out, in) per layer
N_CORES = 8
NPC = N // N_CORES  # teachers per core
P = 128             # partitions = teachers per group
G = NPC // P        # groups per core
# o-chunk per layer: sized so W DMA chunks are ~2-4 MB
OCHUNK = [128, 16, 32, 12]

F32 = mybir.dt.float32
AF = mybir.ActivationFunctionType
ALU = mybir.AluOpType

# Layer 1 output split: o < L1_DVE computed by DVE fused multiply-reduce;
# the last L1_PE columns computed on TensorE from a host-transposed W1
# slice (keeps DVE, the bottleneck engine, under the DMA roofline).
USE_PE = False
L1_PE = 64 if USE_PE else 0
L1_DVE = DIMS[1][0] - L1_PE
L1_CI = DIMS[1][1] // P  # 4 contraction chunks of 128

_cached = {}


def _build_bass():
    nc = bacc.Bacc(trn_type="TRN2", target_bir_lowering=False)

    obs_d = nc.dram_tensor("obs", [NPC, OBS], F32, kind="ExternalInput")
    mean_d = nc.dram_tensor("mean", [NPC, OBS], F32, kind="ExternalInput")
    std_d = nc.dram_tensor("std", [NPC, OBS], F32, kind="ExternalInput")
    W_d, b_d = [], []
    for li, (o, i) in enumerate(DIMS):
        o_dve = L1_DVE if li == 1 else o
        W_d.append(
            nc.dram_tensor(f"W{li}", [NPC, o_dve, i], F32, kind="ExternalInput")
        )
        b_d.append(nc.dram_tensor(f"b{li}", [NPC, o], F32, kind="ExternalInput"))
    # host-transposed slice of W1: [g, ci, i_local(part), teacher, o]
    w1t_d = None
    if USE_PE:
        w1t_d = nc.dram_tensor(
            "W1T", [G, L1_CI, P, P, L1_PE], F32, kind="ExternalInput"
        )
    out_d = nc.dram_tensor("out", [NPC, DIMS[-1][0]], F32, kind="ExternalOutput")

    from concourse.masks import make_identity

    with ExitStack() as ctx:
        tc = ctx.enter_context(tile.TileContext(nc))
        wpool = ctx.enter_context(tc.tile_pool(name="wpool", bufs=5))
        xpool = ctx.enter_context(tc.tile_pool(name="xpool", bufs=3))
        spool = ctx.enter_context(tc.tile_pool(name="spool", bufs=2))
        bpool = ctx.enter_context(tc.tile_pool(name="bpool", bufs=2))
        ppool = ctx.enter_context(tc.tile_pool(name="ppool", bufs=2, space="PSUM"))
        ipool = ctx.enter_context(tc.tile_pool(name="ipool", bufs=1))

        ident = ipool.tile([P, P], F32)
        make_identity(nc, ident)

        def emit_norm(g):
            n0 = g * P

            # ---- obs normalization: x0 = clip((obs - mean)/std, -5, 5) ----
            obs_t = spool.tile([P, OBS], F32, tag="nrm")
            nc.sync.dma_start(out=obs_t, in_=obs_d[n0 : n0 + P, :])
            mean_t = spool.tile([P, OBS], F32, tag="nrm")
            nc.sync.dma_start(out=mean_t, in_=mean_d[n0 : n0 + P, :])
            std_t = spool.tile([P, OBS], F32, tag="nrm")
            nc.sync.dma_start(out=std_t, in_=std_d[n0 : n0 + P, :])

            # Each DVE op may carry at most ONE new semaphore wait (TRN2
            # TT-struct limit), so feed multi-operand ops through
            # single-input ops that absorb the DMA waits first.
            nmean = spool.tile([P, OBS], F32, tag="nmean")
            nc.vector.tensor_scalar_mul(nmean, mean_t, -1.0)
            rstd = spool.tile([P, OBS], F32, tag="rstd")
            nc.vector.reciprocal(rstd, std_t)
            x = xpool.tile([P, OBS], F32, tag="x", name=f"x_in_{g}")
            nc.vector.tensor_add(x, obs_t, nmean)
            nc.vector.tensor_mul(x, x, rstd)
            nc.vector.tensor_scalar(
                out=x, in0=x, scalar1=-5.0, scalar2=5.0,
                op0=ALU.max, op1=ALU.min,
            )
            return x

        def emit_layer(g, li, x):
            n0 = g * P
            O, I = DIMS[li]
            if True:
                bt = bpool.tile([P, O], F32, tag="bias", name=f"b_{g}_{li}")
                nc.sync.dma_start(out=bt, in_=b_d[li][n0 : n0 + P, :])
                y = xpool.tile([P, O], F32, tag="y", name=f"y_{g}_{li}")
                o_dve = L1_DVE if li == 1 else O

                if li == 1 and USE_PE:
                    # TensorE path for y[:, L1_DVE:]: x1 transposed via PE,
                    # then per-teacher matvecs with the host-transposed W1
                    # slice as the stationary operand. ci-outer order keeps
                    # exactly one W1T tile live at a time; each PSUM column
                    # t accumulates across the four ci passes.
                    x1t = xpool.tile([P, L1_CI, P], F32, tag="x1t", name=f"x1t_{g}")
                    for ci in range(L1_CI):
                        pst = ppool.tile([P, P], F32, tag="pst", name=f"pst_{g}_{ci}")
                        nc.tensor.transpose(
                            pst, x[:, ci * P : (ci + 1) * P], ident
                        )
                        nc.scalar.copy(x1t[:, ci, :], pst)
                    yps = ppool.tile([L1_PE, P], F32, tag="yps", name=f"yps_{g}")
                    TH = 32  # teachers per W1T DMA tile
                    for th0 in range(0, P, TH):
                        wtts = []
                        for ci in range(L1_CI):
                            wtt = wpool.tile(
                                [P, TH, L1_PE], F32, tag="w1t", bufs=6,
                                name=f"w1t_{g}_{th0}_{ci}",
                            )
                            # ACT HWDGE ring: these DMAs wait on PE slot
                            # reuse and would stall the SP ring's weight
                            # stream (HWDGE is FIFO per issuing engine).
                            # Emission order guarantees every ELU Exp that
                            # gates DVE progress precedes them in the ACT
                            # stream.
                            nc.scalar.dma_start(
                                out=wtt, in_=w1t_d[g, ci, :, th0 : th0 + TH, :]
                            )
                            wtts.append(wtt)
                        # t-outer, ci-inner: each PSUM column's accumulation
                        # group runs start..stop contiguously (interleaved
                        # groups lose earlier columns' start contributions).
                        for tl in range(TH):
                            t = th0 + tl
                            for ci in range(L1_CI):
                                nc.tensor.matmul(
                                    yps[:, t : t + 1],
                                    lhsT=wtts[ci][:, tl, :],
                                    rhs=x1t[:, ci, t : t + 1],
                                    start=(ci == 0),
                                    stop=(ci == L1_CI - 1),
                                )
                    m1 = xpool.tile([L1_PE, P], F32, tag="m1", name=f"m1_{g}")
                    nc.scalar.copy(m1, yps)
                    pst2 = ppool.tile([P, L1_PE], F32, tag="pst2", name=f"pst2_{g}")
                    nc.tensor.transpose(pst2, m1, ident[:L1_PE, :L1_PE])
                    nc.scalar.copy(y[:, L1_DVE:O], pst2)

                for c0 in range(0, o_dve, OCHUNK[li]):
                    oc = min(OCHUNK[li], o_dve - c0)
                    wt = wpool.tile([P, oc, I], F32, tag="w", name=f"w_{g}_{li}_{c0}")
                    nc.sync.dma_start(
                        out=wt, in_=W_d[li][n0 : n0 + P, c0 : c0 + oc, :]
                    )
                    if I <= 64:
                        # Layer 0: I is tiny, so per-o fused ops are
                        # overhead-dominated. Instead: one in-place batched
                        # multiply (x broadcast across the o dim via a
                        # step-0 AP) + one segmented 3D reduce.
                        x_b = bass.AP(
                            tensor=x.tensor,
                            offset=x.offset,
                            ap=[x.ap[0], [0, oc], x.ap[1]],
                        )
                        nc.vector.tensor_mul(wt, wt, x_b)
                        nc.vector.reduce_sum(
                            out=y[:, c0 : c0 + oc],
                            in_=wt,
                            axis=mybir.AxisListType.X,
                        )
                    else:
                        scr = spool.tile(
                            [P, I], F32, tag="scr", name=f"scr_{g}_{li}_{c0}"
                        )
                        for o in range(oc):
                            # accum_out = sum_i W[o,i]*x[i]  (custom DVE
                            # fused multiply-reduce; the ISA
                            # TENSOR_TENSOR_REDUCE opcode crashes TRN2
                            # hardware on this path)
                            nc.vector.affine_mul_reduce(
                                out=scr,
                                accum_out=y[:, c0 + o : c0 + o + 1],
                                in0=wt[:, o, :],
                                in1=x,
                                scale=1.0,
                                bias=0.0,
                            )
                nc.vector.tensor_add(y, y, bt)
                if li < len(DIMS) - 1:
                    # ELU(y) = exp(min(y,0)) + max(y,0) - 1
                    e = spool.tile([P, O], F32, tag="elu", name=f"e_{g}_{li}")
                    nc.vector.tensor_scalar_min(e, y, 0.0)
                    nc.scalar.activation(e, e, AF.Exp)
                    xn = xpool.tile([P, O], F32, tag="x", name=f"x_{g}_{li}")
                    nc.vector.scalar_tensor_tensor(
                        out=xn, in0=y, scalar=0.0, in1=e,
                        op0=ALU.max, op1=ALU.add,
                    )
                    nc.vector.tensor_scalar_add(xn, xn, -1.0)
                    return xn
                yt = xpool.tile([P, O], F32, tag="yt", name=f"yt_{g}")
                nc.scalar.activation(yt, y, AF.Tanh)
                nc.scalar.dma_start(out=out_d[n0 : n0 + P, :], in_=yt)
                return None

        # Staggered two-group pipeline: group 1 runs one layer behind group
        # 0 so DVE-heavy L0 work overlaps the other group's DMA-heavy L1
        # phase (and the PE matvec phase always has DVE work available).
        for g in range(G):
            xg = emit_norm(g)
            for li in range(len(DIMS)):
                xg = emit_layer(g, li, xg)

    nc.compile()
    return nc


def _get_nc():
    if "nc" not in _cached:
        _cached["nc"] = _build_bass()
    return _cached["nc"]


def _pack_core_inputs(full, c):
    """Shard + lay out one core's inputs (including the transposed W1 slice)."""
    sl = slice(c * NPC, (c + 1) * NPC)
    m = {
        k: np.ascontiguousarray(np.asarray(v)[sl])
        for k, v in full.items()
        if k != "W1"
    }
    w1c = np.asarray(full["W1"])[sl]  # [NPC, 256, 512]
    m["W1"] = np.ascontiguousarray(w1c[:, :L1_DVE, :])
    if USE_PE:
        w1b = w1c[:, L1_DVE:, :]  # [NPC, L1_PE, 512]
        # -> [g, ci, i_local, teacher, o]
        m["W1T"] = np.ascontiguousarray(
            w1b.reshape(G, P, L1_PE, L1_CI, P).transpose(0, 3, 4, 1, 2)
        )
    return m


def kernel(obs, mean, std, W0, b0, W1, b1, W2, b2, W3, b3, _trace=False):
    nc = _get_nc()
    full = {
        "obs": obs, "mean": mean, "std": std,
        "W0": W0, "b0": b0, "W1": W1, "b1": b1,
        "W2": W2, "b2": b2, "W3": W3, "b3": b3,
    }
    in_maps = [_pack_core_inputs(full, c) for c in range(N_CORES)]
    res = run_bass_kernel_spmd(
        nc, in_maps, core_ids=list(range(N_CORES)), trace=_trace
    )
    _cached["last_results"] = res
    out = np.concatenate([res.results[c]["out"] for c in range(N_CORES)], axis=0)
    return out



# revision 6
# speedup vs baseline: 1.6817x; 1.6817x over previous
"""Trainium2 Bass kernel for nn_BatchedTeacherPolicy.

2048 independent per-teacher MLPs (obs-norm -> 48->512->256->128->12,
ELU between layers, tanh at the end). Pure data parallel: 256 teachers
per NeuronCore across 8 cores, 2 groups of 128 teachers per core.

v2 design (vs. the all-DVE baseline):
- Weights are cast to bf16 on the host: halves HBM traffic (the
  roofline for this memory-bound problem) and enables PE fast-weight-
  load + DVE 2x 16-bit modes. rel-err budget is 2e-2; bf16 weight
  quantization contributes ~3e-3.
- L0 stays on DVE in teacher-row space (teacher-on-partition): one
  broadcast multiply + segmented reduce per 128-output chunk.
- L1/L2/L3 run on the (otherwise idle) TensorEngine in transposed
  activation space: activations live as [feature_on_partition,
  teacher_cols]; each teacher's matvec is an LDWEIGHTS(stationary
  W^T chunk, host-pretransposed) + 1-column MATMUL pair, accumulating
  into a [O, 128 teachers] PSUM tile. Layer outputs emerge already in
  the next layer's input layout, so only x1 (DVE->PE handoff) and the
  final y3 need PE transposes.
- ELU(y)+1 = exp(min(y,0)) + max(y,0) is computed instead of ELU; the
  -1 is folded into the next layer's bias on the host
  (b' = b - W_bf16 @ 1), saving one DVE pass per layer.
- DMA is split across both HWDGE rings: SP carries the big col-space
  weight stream, ACT carries row-phase inputs + biases + outputs, so a
  pool-reuse wait on one ring cannot stall the other.
"""

from contextlib import ExitStack

import numpy as np
import ml_dtypes

import concourse.bass as bass
import concourse.bacc as bacc
import concourse.tile as tile
from concourse import mybir
from concourse.bass_utils import run_bass_kernel_spmd

N, OBS = 2048, 48
DIMS = [(512, 48), (256, 512), (128, 256), (12, 128)]  # (out, in) per layer
N_CORES = 8
NPC = N // N_CORES  # teachers per core
P = 128             # partitions = teachers per group
G = NPC // P        # groups per core

O0 = DIMS[0][0]          # 512
O1, I1 = DIMS[1]         # 256, 512
O2, I2 = DIMS[2]         # 128, 256
O3, I3 = DIMS[3]         # 12, 128
CI1 = I1 // P            # 4 contraction chunks for L1
OC1 = O1 // P            # 2 output chunks for L1
CI2 = I2 // P            # 2 contraction chunks for L2
TS1 = 16                 # teachers per W1T DMA tile (8KB/partition)
TS2 = 32                 # teachers per W2T DMA tile (8KB/partition)
OCH0 = 128               # L0 output chunk (DVE path)

F32 = mybir.dt.float32
BF16 = mybir.dt.bfloat16
AF = mybir.ActivationFunctionType
ALU = mybir.AluOpType
NPBF16 = ml_dtypes.bfloat16

_cached = {}


def _build_bass():
    nc = bacc.Bacc(trn_type="TRN2", target_bir_lowering=False)

    obs_d = nc.dram_tensor("obs", [NPC, OBS], F32, kind="ExternalInput")
    mean_d = nc.dram_tensor("mean", [NPC, OBS], F32, kind="ExternalInput")
    std_d = nc.dram_tensor("std", [NPC, OBS], F32, kind="ExternalInput")
    w0_d = nc.dram_tensor("W0", [NPC, O0, OBS], BF16, kind="ExternalInput")
    b0_d = nc.dram_tensor("b0", [NPC, O0], F32, kind="ExternalInput")
    # host-pretransposed weights: [g, ci, i_local(part), teacher, o]
    w1_d = nc.dram_tensor("W1T", [G, CI1, P, P, O1], BF16, kind="ExternalInput")
    b1_d = nc.dram_tensor("b1T", [G, OC1, P, P], F32, kind="ExternalInput")
    w2_d = nc.dram_tensor("W2T", [G, CI2, P, P, O2], BF16, kind="ExternalInput")
    b2_d = nc.dram_tensor("b2T", [G, P, P], F32, kind="ExternalInput")
    w3_d = nc.dram_tensor("W3T", [G, P, P, O3], BF16, kind="ExternalInput")
    b3_d = nc.dram_tensor("b3T", [G, O3, P], F32, kind="ExternalInput")
    out_d = nc.dram_tensor("out", [NPC, O3], F32, kind="ExternalOutput")

    from concourse.masks import make_identity

    with ExitStack() as ctx:
        tc = ctx.enter_context(tile.TileContext(nc))
        w0pool = ctx.enter_context(tc.tile_pool(name="w0pool", bufs=3))
        wcpool = ctx.enter_context(tc.tile_pool(name="wcpool", bufs=6))
        w3pool = ctx.enter_context(tc.tile_pool(name="w3pool", bufs=2))
        xpool = ctx.enter_context(tc.tile_pool(name="xpool", bufs=2))
        spool = ctx.enter_context(tc.tile_pool(name="spool", bufs=3))
        bpool = ctx.enter_context(tc.tile_pool(name="bpool", bufs=2))
        ppool = ctx.enter_context(tc.tile_pool(name="ppool", bufs=1, space="PSUM"))
        ipool = ctx.enter_context(tc.tile_pool(name="ipool", bufs=1))

        ident_h = ipool.tile([P, P], BF16)
        make_identity(nc, ident_h)
        ident_f = ipool.tile([O3, O3], F32)
        make_identity(nc, ident_f)

        def emit_row(g):
            """Row-space phase: norm + L0 on DVE -> x1' = ELU(y0)+1 (bf16).

            Also issues this group's ACT-ring DMAs (obs/std/b0/W0 + the
            col-space biases) and the bias absorb-copies that pre-spend
            the DMA semaphore waits (each DVE op may carry at most one
            new wait on TRN2)."""
            n0 = g * P

            obs_t = spool.tile([P, OBS], F32, tag="nrm", name=f"obs_{g}")
            nc.scalar.dma_start(out=obs_t, in_=obs_d[n0 : n0 + P, :])
            mean_t = spool.tile([P, OBS], F32, tag="nrm", name=f"mean_{g}")
            nc.scalar.dma_start(out=mean_t, in_=mean_d[n0 : n0 + P, :])
            std_t = spool.tile([P, OBS], F32, tag="nrm", name=f"std_{g}")
            nc.scalar.dma_start(out=std_t, in_=std_d[n0 : n0 + P, :])
            b0t = bpool.tile([P, O0], F32, tag="b0", name=f"b0_{g}")
            nc.scalar.dma_start(out=b0t, in_=b0_d[n0 : n0 + P, :])
            # col-space biases (small, fp32, -1 already folded in on host)
            b1t = bpool.tile([P, OC1, P], F32, tag="b1", name=f"b1_{g}")
            for oc in range(OC1):
                nc.scalar.dma_start(out=b1t[:, oc, :], in_=b1_d[g, oc])
            b2t = bpool.tile([P, P], F32, tag="b2", name=f"b2_{g}")
            nc.scalar.dma_start(out=b2t, in_=b2_d[g])
            b3t = bpool.tile([O3, P], F32, tag="b3", name=f"b3_{g}")
            nc.scalar.dma_start(out=b3t, in_=b3_d[g])

            # ---- obs normalization: x0 = clip((obs-mean)/std, -5, 5) ----
            nmean = spool.tile([P, OBS], F32, tag="nmean", name=f"nmean_{g}")
            nc.vector.tensor_scalar_mul(nmean, mean_t, -1.0)
            rstd = spool.tile([P, OBS], F32, tag="rstd", name=f"rstd_{g}")
            nc.vector.reciprocal(rstd, std_t)
            x0 = spool.tile([P, OBS], F32, tag="x0", name=f"x0_{g}")
            nc.vector.tensor_add(x0, obs_t, nmean)
            nc.vector.tensor_mul(x0, x0, rstd)
            x0h = xpool.tile([P, OBS], BF16, tag="x0h", name=f"x0h_{g}")
            nc.vector.tensor_scalar(
                out=x0h, in0=x0, scalar1=-5.0, scalar2=5.0,
                op0=ALU.max, op1=ALU.min,
            )

            # ---- L0: y0[t, o] = sum_i W0[t, o, i] * x0[t, i] ----
            y0 = xpool.tile([P, O0], F32, tag="y0", name=f"y0_{g}")
            for c0 in range(0, O0, OCH0):
                wt = w0pool.tile([P, OCH0, OBS], BF16, tag="w0", name=f"w0_{g}_{c0}")
                nc.scalar.dma_start(out=wt, in_=w0_d[n0 : n0 + P, c0 : c0 + OCH0, :])
                x_b = bass.AP(
                    tensor=x0h.tensor,
                    offset=x0h.offset,
                    ap=[x0h.ap[0], [0, OCH0], x0h.ap[1]],
                )
                nc.vector.tensor_mul(wt, wt, x_b)
                nc.vector.reduce_sum(
                    out=y0[:, c0 : c0 + OCH0], in_=wt, axis=mybir.AxisListType.X
                )
            nc.vector.tensor_add(y0, y0, b0t)
            # x1' = ELU(y0)+1 = exp(min(y0,0)) + max(y0,0), in bf16
            e0 = spool.tile([P, O0], F32, tag="e0", name=f"e0_{g}")
            nc.vector.tensor_scalar_min(e0, y0, 0.0)
            nc.scalar.activation(e0, e0, AF.Exp)
            x1h = xpool.tile([P, O0], BF16, tag="x1h", name=f"x1h_{g}")
            nc.vector.scalar_tensor_tensor(
                out=x1h, in0=y0, scalar=0.0, in1=e0, op0=ALU.max, op1=ALU.add,
            )

            # absorb the bias-DMA waits now so the ELU epilogue ops in the
            # col phase carry only their PSUM wait
            b1a = bpool.tile([P, OC1, P], F32, tag="b1a", name=f"b1a_{g}")
            nc.vector.tensor_scalar_mul(b1a, b1t, 1.0)
            b2a = bpool.tile([P, P], F32, tag="b2a", name=f"b2a_{g}")
            nc.vector.tensor_scalar_mul(b2a, b2t, 1.0)
            b3a = bpool.tile([O3, P], F32, tag="b3a", name=f"b3a_{g}")
            nc.vector.tensor_scalar_mul(b3a, b3t, 1.0)
            return x1h, b1a, b2a, b3a

        def elu_chunk(yps, babs, out_ap, g, li, oc):
            """out = exp(min(y,0)) + max(y,0) with y = psum + bias, bf16."""
            t0 = spool.tile([P, P], F32, tag="t0", name=f"t0_{g}_{li}_{oc}")
            nc.vector.tensor_add(t0, yps, babs)
            e = spool.tile([P, P], F32, tag="el", name=f"el_{g}_{li}_{oc}")
            nc.vector.tensor_scalar_min(e, t0, 0.0)
            nc.scalar.activation(e, e, AF.Exp)
            nc.vector.scalar_tensor_tensor(
                out=out_ap, in0=t0, scalar=0.0, in1=e, op0=ALU.max, op1=ALU.add,
            )

        def emit_col(g, x1h, b1a, b2a, b3a):
            """Column-space phase on PE: transpose x1, then L1/L2/L3 as
            per-teacher LDWEIGHTS+MATMUL pairs, epilogues on DVE/ACT."""
            n0 = g * P

            # x1' [t, 512] -> x1T [ci][i, t] (bf16) via PE transposes
            x1T = xpool.tile([P, CI1, P], BF16, tag="x1T", name=f"x1T_{g}")
            for ci in range(CI1):
                pst = ppool.tile([P, P], BF16, tag="pst", bufs=2, name=f"pst_{g}_{ci}")
                nc.tensor.transpose(pst, x1h[:, ci * P : (ci + 1) * P], ident_h)
                nc.vector.tensor_copy(x1T[:, ci, :], pst)

            # ---- L1 ----
            yps1 = [
                ppool.tile([P, P], F32, tag=f"yps1_{oc}", name=f"yps1_{g}_{oc}")
                for oc in range(OC1)
            ]
            for tb in range(0, P, TS1):
                wts = []
                for ci in range(CI1):
                    wt = wcpool.tile(
                        [P, TS1, O1], BF16, tag="wc", name=f"w1_{g}_{tb}_{ci}"
                    )
                    nc.sync.dma_start(out=wt, in_=w1_d[g, ci, :, tb : tb + TS1, :])
                    wts.append(wt)
                for tl in range(TS1):
                    t = tb + tl
                    for oc in range(OC1):
                        for ci in range(CI1):
                            nc.tensor.matmul(
                                yps1[oc][:, t : t + 1],
                                lhsT=wts[ci][:, tl, oc * P : (oc + 1) * P],
                                rhs=x1T[:, ci, t : t + 1],
                                start=(ci == 0),
                                stop=(ci == CI1 - 1),
                            )
            x2T = xpool.tile([P, CI2, P], BF16, tag="x2T", name=f"x2T_{g}")
            for oc in range(OC1):
                elu_chunk(yps1[oc], b1a[:, oc, :], x2T[:, oc, :], g, 1, oc)

            # ---- L2 ----
            yps2 = ppool.tile([P, P], F32, tag="yps2", name=f"yps2_{g}")
            for tb in range(0, P, TS2):
                wts = []
                for ci in range(CI2):
                    wt = wcpool.tile(
                        [P, TS2, O2], BF16, tag="wc", name=f"w2_{g}_{tb}_{ci}"
                    )
                    nc.sync.dma_start(out=wt, in_=w2_d[g, ci, :, tb : tb + TS2, :])
                    wts.append(wt)
                for tl in range(TS2):
                    t = tb + tl
                    for ci in range(CI2):
                        nc.tensor.matmul(
                            yps2[:, t : t + 1],
                            lhsT=wts[ci][:, tl, :],
                            rhs=x2T[:, ci, t : t + 1],
                            start=(ci == 0),
                            stop=(ci == CI2 - 1),
                        )
            x3T = xpool.tile([P, P], BF16, tag="x3T", name=f"x3T_{g}")
            elu_chunk(yps2, b2a, x3T, g, 2, 0)

            # ---- L3 ----
            w3t = w3pool.tile([P, P, O3], BF16, tag="w3", name=f"w3_{g}")
            nc.sync.dma_start(out=w3t, in_=w3_d[g])
            yps3 = ppool.tile([O3, P], F32, tag="yps3", name=f"yps3_{g}")
            for t in range(P):
                nc.tensor.matmul(
                    yps3[:, t : t + 1],
                    lhsT=w3t[:, t, :],
                    rhs=x3T[:, t : t + 1],
                    start=True,
                    stop=True,
                )
            y3 = spool.tile([O3, P], F32, tag="y3", name=f"y3_{g}")
            nc.vector.tensor_add(y3, yps3, b3a)
            nc.scalar.activation(y3, y3, AF.Tanh)
            pso = ppool.tile([P, O3], F32, tag="pso", name=f"pso_{g}")
            nc.tensor.transpose(pso, y3, ident_f)
            yt = spool.tile([P, O3], F32, tag="yt", name=f"yt_{g}")
            nc.scalar.copy(yt, pso)
            nc.scalar.dma_start(out=out_d[n0 : n0 + P, :], in_=yt)

        rows = [emit_row(g) for g in range(G)]
        for g in range(G):
            emit_col(g, *rows[g])

    nc.compile()
    return nc


def _get_nc():
    if "nc" not in _cached:
        _cached["nc"] = _build_bass()
    return _cached["nc"]


def _pack_core_inputs(full, c):
    """Shard + lay out one core's inputs (bf16 weights, pretransposed)."""
    sl = slice(c * NPC, (c + 1) * NPC)
    f32 = np.float32
    w0 = np.asarray(full["W0"])[sl].astype(NPBF16)          # [NPC, 512, 48]
    w1 = np.asarray(full["W1"])[sl].astype(NPBF16)          # [NPC, 256, 512]
    w2 = np.asarray(full["W2"])[sl].astype(NPBF16)          # [NPC, 128, 256]
    w3 = np.asarray(full["W3"])[sl].astype(NPBF16)          # [NPC, 12, 128]
    # fold the ELU "-1" into the next layer's bias: b' = b - W_bf16 @ 1
    b1p = np.asarray(full["b1"])[sl] - w1.astype(f32).sum(-1)
    b2p = np.asarray(full["b2"])[sl] - w2.astype(f32).sum(-1)
    b3p = np.asarray(full["b3"])[sl] - w3.astype(f32).sum(-1)
    # W1T[g, ci, i, t, oc*128+o] = W1[g*128+t, oc*128+o, ci*128+i]
    w1t = np.ascontiguousarray(
        w1.reshape(G, P, OC1, P, CI1, P).transpose(0, 4, 5, 1, 2, 3)
        .reshape(G, CI1, P, P, O1)
    )
    b1t = np.ascontiguousarray(
        b1p.reshape(G, P, OC1, P).transpose(0, 2, 3, 1).astype(f32)
    )
    w2t = np.ascontiguousarray(
        w2.reshape(G, P, P, CI2, P).transpose(0, 3, 4, 1, 2)
    )
    b2t = np.ascontiguousarray(b2p.reshape(G, P, P).transpose(0, 2, 1).astype(f32))
    w3t = np.ascontiguousarray(w3.reshape(G, P, O3, P).transpose(0, 3, 1, 2))
    b3t = np.ascontiguousarray(b3p.reshape(G, P, O3).transpose(0, 2, 1).astype(f32))
    return {
        "obs": np.ascontiguousarray(np.asarray(full["obs"])[sl]),
        "mean": np.ascontiguousarray(np.asarray(full["mean"])[sl]),
        "std": np.ascontiguousarray(np.asarray(full["std"])[sl]),
        "W0": np.ascontiguousarray(w0),
        "b0": np.ascontiguousarray(np.asarray(full["b0"])[sl]),
        "W1T": w1t, "b1T": b1t,
        "W2T": w2t, "b2T": b2t,
        "W3T": w3t, "b3T": b3t,
    }


def kernel(obs, mean, std, W0, b0, W1, b1, W2, b2, W3, b3, _trace=False):
    nc = _get_nc()
    full = {
        "obs": obs, "mean": mean, "std": std,
        "W0": W0, "b0": b0, "W1": W1, "b1": b1,
        "W2": W2, "b2": b2, "W3": W3, "b3": b3,
    }
    in_maps = [_pack_core_inputs(full, c) for c in range(N_CORES)]
    res = run_bass_kernel_spmd(
        nc, in_maps, core_ids=list(range(N_CORES)), trace=_trace
    )
    _cached["last_results"] = res
    out = np.concatenate([res.results[c]["out"] for c in range(N_CORES)], axis=0)
    return out


# revision 10
# speedup vs baseline: 1.9075x; 1.1343x over previous
"""Trainium2 Bass kernel for nn_BatchedTeacherPolicy.

2048 independent per-teacher MLPs (obs-norm -> 48->512->256->128->12,
ELU between layers, tanh at the end). Pure data parallel: 256 teachers
per NeuronCore across 8 cores, 2 groups of 128 teachers per core.

v2 design (vs. the all-DVE baseline):
- Weights are cast to bf16 on the host: halves HBM traffic (the
  roofline for this memory-bound problem) and enables PE fast-weight-
  load + DVE 2x 16-bit modes. rel-err budget is 2e-2; bf16 weight
  quantization contributes ~3e-3.
- L0 stays on DVE in teacher-row space (teacher-on-partition): one
  broadcast multiply + segmented reduce per 128-output chunk.
- L1/L2/L3 run on the (otherwise idle) TensorEngine in transposed
  activation space: activations live as [feature_on_partition,
  teacher_cols]; each teacher's matvec is an LDWEIGHTS(stationary
  W^T chunk, host-pretransposed) + 1-column MATMUL pair, accumulating
  into a [O, 128 teachers] PSUM tile. Layer outputs emerge already in
  the next layer's input layout, so only x1 (DVE->PE handoff) and the
  final y3 need PE transposes.
- ELU(y)+1 = exp(min(y,0)) + max(y,0) is computed instead of ELU; the
  -1 is folded into the next layer's bias on the host
  (b' = b - W_bf16 @ 1), saving one DVE pass per layer.
- DMA is split across both HWDGE rings: SP carries the big col-space
  weight stream, ACT carries row-phase inputs + biases + outputs, so a
  pool-reuse wait on one ring cannot stall the other.
"""

from contextlib import ExitStack

import numpy as np
import ml_dtypes

import concourse.bass as bass
import concourse.bacc as bacc
import concourse.tile as tile
from concourse import mybir
from concourse.bass_utils import run_bass_kernel_spmd

N, OBS = 2048, 48
DIMS = [(512, 48), (256, 512), (128, 256), (12, 128)]  # (out, in) per layer
N_CORES = 8
NPC = N // N_CORES  # teachers per core
P = 128             # partitions = teachers per group
G = NPC // P        # groups per core

O0 = DIMS[0][0]          # 512
O1, I1 = DIMS[1]         # 256, 512
O2, I2 = DIMS[2]         # 128, 256
O3, I3 = DIMS[3]         # 12, 128
CI1 = I1 // P            # 4 contraction chunks for L1
OC1 = O1 // P            # 2 output chunks for L1
CI2 = I2 // P            # 2 contraction chunks for L2
TS1 = 16                 # teachers per W1T DMA tile (8KB/partition)
TS2 = 32                 # teachers per W2T DMA tile (8KB/partition)
OCH0 = 128               # L0 output chunk (DVE path)

F32 = mybir.dt.float32
BF16 = mybir.dt.bfloat16
AF = mybir.ActivationFunctionType
ALU = mybir.AluOpType
NPBF16 = ml_dtypes.bfloat16

_cached = {}


def _build_bass():
    nc = bacc.Bacc(trn_type="TRN2", target_bir_lowering=False)

    obs_d = nc.dram_tensor("obs", [NPC, OBS], F32, kind="ExternalInput")
    mean_d = nc.dram_tensor("mean", [NPC, OBS], F32, kind="ExternalInput")
    std_d = nc.dram_tensor("std", [NPC, OBS], F32, kind="ExternalInput")
    w0_d = nc.dram_tensor("W0", [NPC, O0, OBS], BF16, kind="ExternalInput")
    b0_d = nc.dram_tensor("b0", [NPC, O0], F32, kind="ExternalInput")
    # host-pretransposed weights: [g, ci, i_local(part), teacher, o]
    w1_d = nc.dram_tensor("W1T", [G, CI1, P, P, O1], BF16, kind="ExternalInput")
    b1_d = nc.dram_tensor("b1T", [G, OC1, P, P], F32, kind="ExternalInput")
    w2_d = nc.dram_tensor("W2T", [G, CI2, P, P, O2], BF16, kind="ExternalInput")
    b2_d = nc.dram_tensor("b2T", [G, P, P], F32, kind="ExternalInput")
    w3_d = nc.dram_tensor("W3T", [G, P, P, O3], BF16, kind="ExternalInput")
    b3_d = nc.dram_tensor("b3T", [G, O3, P], F32, kind="ExternalInput")
    out_d = nc.dram_tensor("out", [NPC, O3], F32, kind="ExternalOutput")

    from concourse.masks import make_identity

    with ExitStack() as ctx:
        tc = ctx.enter_context(tile.TileContext(nc))
        w0pool = ctx.enter_context(tc.tile_pool(name="w0pool", bufs=4))
        wcpool = ctx.enter_context(tc.tile_pool(name="wcpool", bufs=12))
        w3pool = ctx.enter_context(tc.tile_pool(name="w3pool", bufs=2))
        xpool = ctx.enter_context(tc.tile_pool(name="xpool", bufs=2))
        spool = ctx.enter_context(tc.tile_pool(name="spool", bufs=3))
        bpool = ctx.enter_context(tc.tile_pool(name="bpool", bufs=2))
        ppool = ctx.enter_context(tc.tile_pool(name="ppool", bufs=1, space="PSUM"))
        ipool = ctx.enter_context(tc.tile_pool(name="ipool", bufs=1))

        ident_h = ipool.tile([P, P], BF16)
        make_identity(nc, ident_h)
        ident_f = ipool.tile([O3, O3], F32)
        make_identity(nc, ident_f)

        def emit_row_dmas(g):
            """Issue this group's row-phase DMAs on the ACT ring. Emitted
            for ALL groups before any ACT compute op so a compute wait
            (e.g. the ELU Exp gated on L0) can never head-of-line block a
            later group's input stream."""
            n0 = g * P

            obs_t = spool.tile([P, OBS], F32, tag="nrm", bufs=6, name=f"obs_{g}")
            nc.scalar.dma_start(out=obs_t, in_=obs_d[n0 : n0 + P, :])
            mean_t = spool.tile([P, OBS], F32, tag="nrm", bufs=6, name=f"mean_{g}")
            nc.scalar.dma_start(out=mean_t, in_=mean_d[n0 : n0 + P, :])
            std_t = spool.tile([P, OBS], F32, tag="nrm", bufs=6, name=f"std_{g}")
            nc.scalar.dma_start(out=std_t, in_=std_d[n0 : n0 + P, :])
            b0t = bpool.tile([P, O0], F32, tag="b0", name=f"b0_{g}")
            nc.scalar.dma_start(out=b0t, in_=b0_d[n0 : n0 + P, :])
            # col-space biases (small, fp32, -1 already folded in on host)
            b1t = bpool.tile([P, OC1, P], F32, tag="b1", name=f"b1_{g}")
            for oc in range(OC1):
                nc.scalar.dma_start(out=b1t[:, oc, :], in_=b1_d[g, oc])
            b2t = bpool.tile([P, P], F32, tag="b2", name=f"b2_{g}")
            nc.scalar.dma_start(out=b2t, in_=b2_d[g])
            b3t = bpool.tile([O3, P], F32, tag="b3", name=f"b3_{g}")
            nc.scalar.dma_start(out=b3t, in_=b3_d[g])
            w0ts = []
            for c0 in range(0, O0, OCH0):
                wt = w0pool.tile([P, OCH0, OBS], BF16, tag="w0", name=f"w0_{g}_{c0}")
                nc.scalar.dma_start(out=wt, in_=w0_d[n0 : n0 + P, c0 : c0 + OCH0, :])
                w0ts.append(wt)
            return obs_t, mean_t, std_t, b0t, b1t, b2t, b3t, w0ts

        def emit_row_compute(g, dmas):
            """Row-space phase: norm + L0 on DVE -> x1' = ELU(y0)+1 (bf16)."""
            obs_t, mean_t, std_t, b0t, b1t, b2t, b3t, w0ts = dmas

            # ---- obs normalization: x0 = clip((obs-mean)/std, -5, 5) ----
            nmean = spool.tile([P, OBS], F32, tag="nmean", name=f"nmean_{g}")
            nc.vector.tensor_scalar_mul(nmean, mean_t, -1.0)
            rstd = spool.tile([P, OBS], F32, tag="rstd", name=f"rstd_{g}")
            nc.vector.reciprocal(rstd, std_t)
            x0 = spool.tile([P, OBS], F32, tag="x0", name=f"x0_{g}")
            nc.vector.tensor_add(x0, obs_t, nmean)
            nc.vector.tensor_mul(x0, x0, rstd)
            x0h = xpool.tile([P, OBS], BF16, tag="x0h", name=f"x0h_{g}")
            nc.vector.tensor_scalar(
                out=x0h, in0=x0, scalar1=-5.0, scalar2=5.0,
                op0=ALU.max, op1=ALU.min,
            )

            # ---- L0: y0[t, o] = sum_i W0[t, o, i] * x0[t, i] ----
            y0 = xpool.tile([P, O0], F32, tag="y0", name=f"y0_{g}")
            for ch, c0 in enumerate(range(0, O0, OCH0)):
                wt = w0ts[ch]
                x_b = bass.AP(
                    tensor=x0h.tensor,
                    offset=x0h.offset,
                    ap=[x0h.ap[0], [0, OCH0], x0h.ap[1]],
                )
                nc.vector.tensor_mul(wt, wt, x_b)
                nc.vector.reduce_sum(
                    out=y0[:, c0 : c0 + OCH0], in_=wt, axis=mybir.AxisListType.X
                )
            nc.vector.tensor_add(y0, y0, b0t)
            # x1' = ELU(y0)+1 = exp(min(y0,0)) + max(y0,0), in bf16
            e0 = spool.tile([P, O0], F32, tag="e0", name=f"e0_{g}")
            nc.vector.tensor_scalar_min(e0, y0, 0.0)
            nc.scalar.activation(e0, e0, AF.Exp)
            x1h = xpool.tile([P, O0], BF16, tag="x1h", name=f"x1h_{g}")
            nc.vector.scalar_tensor_tensor(
                out=x1h, in0=y0, scalar=0.0, in1=e0, op0=ALU.max, op1=ALU.add,
            )

            # absorb the bias-DMA waits now so the ELU epilogue ops in the
            # col phase carry only their PSUM wait
            b1a = bpool.tile([P, OC1, P], F32, tag="b1a", name=f"b1a_{g}")
            nc.vector.tensor_scalar_mul(b1a, b1t, 1.0)
            b2a = bpool.tile([P, P], F32, tag="b2a", name=f"b2a_{g}")
            nc.vector.tensor_scalar_mul(b2a, b2t, 1.0)
            b3a = bpool.tile([O3, P], F32, tag="b3a", name=f"b3a_{g}")
            nc.vector.tensor_scalar_mul(b3a, b3t, 1.0)
            return x1h, b1a, b2a, b3a

        def elu_chunk(yps, babs, out_ap, g, li, oc):
            """out = exp(min(y,0)) + max(y,0) with y = psum + bias, bf16."""
            t0 = spool.tile([P, P], F32, tag="t0", name=f"t0_{g}_{li}_{oc}")
            nc.vector.tensor_add(t0, yps, babs)
            e = spool.tile([P, P], F32, tag="el", name=f"el_{g}_{li}_{oc}")
            nc.vector.tensor_scalar_min(e, t0, 0.0)
            nc.scalar.activation(e, e, AF.Exp)
            nc.vector.scalar_tensor_tensor(
                out=out_ap, in0=t0, scalar=0.0, in1=e, op0=ALU.max, op1=ALU.add,
            )

        def emit_col(g, x1h, b1a, b2a, b3a):
            """Column-space phase on PE: transpose x1, then L1/L2/L3 as
            per-teacher LDWEIGHTS+MATMUL pairs, epilogues on DVE/ACT."""
            n0 = g * P

            # x1' [t, 512] -> x1T [ci][i, t] (bf16) via PE transposes
            x1T = xpool.tile([P, CI1, P], BF16, tag="x1T", name=f"x1T_{g}")
            for ci in range(CI1):
                pst = ppool.tile([P, P], BF16, tag="pst", bufs=2, name=f"pst_{g}_{ci}")
                nc.tensor.transpose(pst, x1h[:, ci * P : (ci + 1) * P], ident_h)
                nc.vector.tensor_copy(x1T[:, ci, :], pst)

            # ---- L1 ----
            yps1 = [
                ppool.tile([P, P], F32, tag=f"yps1_{oc}", name=f"yps1_{g}_{oc}")
                for oc in range(OC1)
            ]
            for tb in range(0, P, TS1):
                wts = []
                for ci in range(CI1):
                    wt = wcpool.tile(
                        [P, TS1, O1], BF16, tag="wc", name=f"w1_{g}_{tb}_{ci}"
                    )
                    nc.sync.dma_start(out=wt, in_=w1_d[g, ci, :, tb : tb + TS1, :])
                    wts.append(wt)
                for tl in range(TS1):
                    t = tb + tl
                    for oc in range(OC1):
                        for ci in range(CI1):
                            nc.tensor.matmul(
                                yps1[oc][:, t : t + 1],
                                lhsT=wts[ci][:, tl, oc * P : (oc + 1) * P],
                                rhs=x1T[:, ci, t : t + 1],
                                start=(ci == 0),
                                stop=(ci == CI1 - 1),
                            )
            x2T = xpool.tile([P, CI2, P], BF16, tag="x2T", name=f"x2T_{g}")
            for oc in range(OC1):
                elu_chunk(yps1[oc], b1a[:, oc, :], x2T[:, oc, :], g, 1, oc)

            # ---- L2 ----
            yps2 = ppool.tile([P, P], F32, tag="yps2", name=f"yps2_{g}")
            for tb in range(0, P, TS2):
                wts = []
                for ci in range(CI2):
                    wt = wcpool.tile(
                        [P, TS2, O2], BF16, tag="wc", name=f"w2_{g}_{tb}_{ci}"
                    )
                    nc.sync.dma_start(out=wt, in_=w2_d[g, ci, :, tb : tb + TS2, :])
                    wts.append(wt)
                for tl in range(TS2):
                    t = tb + tl
                    for ci in range(CI2):
                        nc.tensor.matmul(
                            yps2[:, t : t + 1],
                            lhsT=wts[ci][:, tl, :],
                            rhs=x2T[:, ci, t : t + 1],
                            start=(ci == 0),
                            stop=(ci == CI2 - 1),
                        )
            x3T = xpool.tile([P, P], BF16, tag="x3T", name=f"x3T_{g}")
            elu_chunk(yps2, b2a, x3T, g, 2, 0)

            # ---- L3 ----
            w3t = w3pool.tile([P, P, O3], BF16, tag="w3", name=f"w3_{g}")
            nc.sync.dma_start(out=w3t, in_=w3_d[g])
            yps3 = ppool.tile([O3, P], F32, tag="yps3", name=f"yps3_{g}")
            for t in range(P):
                nc.tensor.matmul(
                    yps3[:, t : t + 1],
                    lhsT=w3t[:, t, :],
                    rhs=x3T[:, t : t + 1],
                    start=True,
                    stop=True,
                )
            y3 = spool.tile([O3, P], F32, tag="y3", name=f"y3_{g}")
            nc.vector.tensor_add(y3, yps3, b3a)
            nc.scalar.activation(y3, y3, AF.Tanh)
            pso = ppool.tile([P, O3], F32, tag="pso", name=f"pso_{g}")
            nc.tensor.transpose(pso, y3, ident_f)
            yt = spool.tile([P, O3], F32, tag="yt", name=f"yt_{g}")
            nc.scalar.copy(yt, pso)
            nc.scalar.dma_start(out=out_d[n0 : n0 + P, :], in_=yt)

        dmas = [emit_row_dmas(g) for g in range(G)]
        rows = [emit_row_compute(g, dmas[g]) for g in range(G)]
        for g in range(G):
            emit_col(g, *rows[g])

    nc.compile()
    return nc


def _get_nc():
    if "nc" not in _cached:
        _cached["nc"] = _build_bass()
    return _cached["nc"]


def _pack_core_inputs(full, c):
    """Shard + lay out one core's inputs (bf16 weights, pretransposed)."""
    sl = slice(c * NPC, (c + 1) * NPC)
    f32 = np.float32
    w0 = np.asarray(full["W0"])[sl].astype(NPBF16)          # [NPC, 512, 48]
    w1 = np.asarray(full["W1"])[sl].astype(NPBF16)          # [NPC, 256, 512]
    w2 = np.asarray(full["W2"])[sl].astype(NPBF16)          # [NPC, 128, 256]
    w3 = np.asarray(full["W3"])[sl].astype(NPBF16)          # [NPC, 12, 128]
    # fold the ELU "-1" into the next layer's bias: b' = b - W_bf16 @ 1
    b1p = np.asarray(full["b1"])[sl] - w1.astype(f32).sum(-1)
    b2p = np.asarray(full["b2"])[sl] - w2.astype(f32).sum(-1)
    b3p = np.asarray(full["b3"])[sl] - w3.astype(f32).sum(-1)
    # W1T[g, ci, i, t, oc*128+o] = W1[g*128+t, oc*128+o, ci*128+i]
    w1t = np.ascontiguousarray(
        w1.reshape(G, P, OC1, P, CI1, P).transpose(0, 4, 5, 1, 2, 3)
        .reshape(G, CI1, P, P, O1)
    )
    b1t = np.ascontiguousarray(
        b1p.reshape(G, P, OC1, P).transpose(0, 2, 3, 1).astype(f32)
    )
    w2t = np.ascontiguousarray(
        w2.reshape(G, P, P, CI2, P).transpose(0, 3, 4, 1, 2)
    )
    b2t = np.ascontiguousarray(b2p.reshape(G, P, P).transpose(0, 2, 1).astype(f32))
    w3t = np.ascontiguousarray(w3.reshape(G, P, O3, P).transpose(0, 3, 1, 2))
    b3t = np.ascontiguousarray(b3p.reshape(G, P, O3).transpose(0, 2, 1).astype(f32))
    return {
        "obs": np.ascontiguousarray(np.asarray(full["obs"])[sl]),
        "mean": np.ascontiguousarray(np.asarray(full["mean"])[sl]),
        "std": np.ascontiguousarray(np.asarray(full["std"])[sl]),
        "W0": np.ascontiguousarray(w0),
        "b0": np.ascontiguousarray(np.asarray(full["b0"])[sl]),
        "W1T": w1t, "b1T": b1t,
        "W2T": w2t, "b2T": b2t,
        "W3T": w3t, "b3T": b3t,
    }


def kernel(obs, mean, std, W0, b0, W1, b1, W2, b2, W3, b3, _trace=False):
    nc = _get_nc()
    full = {
        "obs": obs, "mean": mean, "std": std,
        "W0": W0, "b0": b0, "W1": W1, "b1": b1,
        "W2": W2, "b2": b2, "W3": W3, "b3": b3,
    }
    in_maps = [_pack_core_inputs(full, c) for c in range(N_CORES)]
    res = run_bass_kernel_spmd(
        nc, in_maps, core_ids=list(range(N_CORES)), trace=_trace
    )
    _cached["last_results"] = res
    out = np.concatenate([res.results[c]["out"] for c in range(N_CORES)], axis=0)
    return out


# revision 13
# speedup vs baseline: 1.9525x; 1.0236x over previous
"""Trainium2 Bass kernel for nn_BatchedTeacherPolicy.

2048 independent per-teacher MLPs (obs-norm -> 48->512->256->128->12,
ELU between layers, tanh at the end). Pure data parallel: 256 teachers
per NeuronCore across 8 cores, 2 groups of 128 teachers per core.

v2 design (vs. the all-DVE baseline):
- Weights are cast to bf16 on the host: halves HBM traffic (the
  roofline for this memory-bound problem) and enables PE fast-weight-
  load + DVE 2x 16-bit modes. rel-err budget is 2e-2; bf16 weight
  quantization contributes ~3e-3.
- L0 stays on DVE in teacher-row space (teacher-on-partition): one
  broadcast multiply + segmented reduce per 128-output chunk.
- L1/L2/L3 run on the (otherwise idle) TensorEngine in transposed
  activation space: activations live as [feature_on_partition,
  teacher_cols]; each teacher's matvec is an LDWEIGHTS(stationary
  W^T chunk, host-pretransposed) + 1-column MATMUL pair, accumulating
  into a [O, 128 teachers] PSUM tile. Layer outputs emerge already in
  the next layer's input layout, so only x1 (DVE->PE handoff) and the
  final y3 need PE transposes.
- ELU(y)+1 = exp(min(y,0)) + max(y,0) is computed instead of ELU; the
  -1 is folded into the next layer's bias on the host
  (b' = b - W_bf16 @ 1), saving one DVE pass per layer.
- DMA is split across both HWDGE rings: SP carries the big col-space
  weight stream, ACT carries row-phase inputs + biases + outputs, so a
  pool-reuse wait on one ring cannot stall the other.
"""

from contextlib import ExitStack

import numpy as np
import ml_dtypes

import concourse.bass as bass
import concourse.bacc as bacc
import concourse.tile as tile
from concourse import mybir
from concourse.bass_utils import run_bass_kernel_spmd

N, OBS = 2048, 48
DIMS = [(512, 48), (256, 512), (128, 256), (12, 128)]  # (out, in) per layer
N_CORES = 8
NPC = N // N_CORES  # teachers per core
P = 128             # partitions = teachers per group
G = NPC // P        # groups per core

O0 = DIMS[0][0]          # 512
O1, I1 = DIMS[1]         # 256, 512
O2, I2 = DIMS[2]         # 128, 256
O3, I3 = DIMS[3]         # 12, 128
CI1 = I1 // P            # 4 contraction chunks for L1
OC1 = O1 // P            # 2 output chunks for L1
CI2 = I2 // P            # 2 contraction chunks for L2
TS1 = 16                 # teachers per W1T DMA tile (8KB/partition)
TS2 = 32                 # teachers per W2T DMA tile (8KB/partition)
OCH0 = 128               # L0 output chunk (DVE path)

F32 = mybir.dt.float32
BF16 = mybir.dt.bfloat16
AF = mybir.ActivationFunctionType
ALU = mybir.AluOpType
NPBF16 = ml_dtypes.bfloat16

_cached = {}


def _build_bass():
    nc = bacc.Bacc(trn_type="TRN2", target_bir_lowering=False)

    obs_d = nc.dram_tensor("obs", [NPC, OBS], F32, kind="ExternalInput")
    mean_d = nc.dram_tensor("mean", [NPC, OBS], F32, kind="ExternalInput")
    std_d = nc.dram_tensor("std", [NPC, OBS], F32, kind="ExternalInput")
    w0_d = nc.dram_tensor("W0", [NPC, O0, OBS], BF16, kind="ExternalInput")
    b0_d = nc.dram_tensor("b0", [NPC, O0], F32, kind="ExternalInput")
    # host-pretransposed weights: [g, ci, i_local(part), teacher, o]
    w1_d = nc.dram_tensor("W1T", [G, CI1, P, P, O1], BF16, kind="ExternalInput")
    b1_d = nc.dram_tensor("b1T", [G, OC1, P, P], F32, kind="ExternalInput")
    w2_d = nc.dram_tensor("W2T", [G, CI2, P, P, O2], BF16, kind="ExternalInput")
    b2_d = nc.dram_tensor("b2T", [G, P, P], F32, kind="ExternalInput")
    w3_d = nc.dram_tensor("W3T", [G, P, P, O3], BF16, kind="ExternalInput")
    b3_d = nc.dram_tensor("b3T", [G, O3, P], F32, kind="ExternalInput")
    out_d = nc.dram_tensor("out", [NPC, O3], F32, kind="ExternalOutput")

    from concourse.masks import make_identity

    with ExitStack() as ctx:
        tc = ctx.enter_context(tile.TileContext(nc))
        w0pool = ctx.enter_context(tc.tile_pool(name="w0pool", bufs=4))
        wcpool = ctx.enter_context(tc.tile_pool(name="wcpool", bufs=12))
        w3pool = ctx.enter_context(tc.tile_pool(name="w3pool", bufs=2))
        xpool = ctx.enter_context(tc.tile_pool(name="xpool", bufs=2))
        spool = ctx.enter_context(tc.tile_pool(name="spool", bufs=3))
        bpool = ctx.enter_context(tc.tile_pool(name="bpool", bufs=2))
        ppool = ctx.enter_context(tc.tile_pool(name="ppool", bufs=1, space="PSUM"))
        ipool = ctx.enter_context(tc.tile_pool(name="ipool", bufs=1))

        ident_h = ipool.tile([P, P], BF16)
        make_identity(nc, ident_h)
        ident_f = ipool.tile([O3, O3], F32)
        make_identity(nc, ident_f)

        def emit_row_dmas(g):
            """Issue this group's row-phase DMAs on the ACT ring. Emitted
            for ALL groups before any ACT compute op so a compute wait
            (e.g. the ELU Exp gated on L0) can never head-of-line block a
            later group's input stream."""
            n0 = g * P

            obs_t = spool.tile([P, OBS], F32, tag="nrm", bufs=6, name=f"obs_{g}")
            nc.scalar.dma_start(out=obs_t, in_=obs_d[n0 : n0 + P, :])
            mean_t = spool.tile([P, OBS], F32, tag="nrm", bufs=6, name=f"mean_{g}")
            nc.scalar.dma_start(out=mean_t, in_=mean_d[n0 : n0 + P, :])
            std_t = spool.tile([P, OBS], F32, tag="nrm", bufs=6, name=f"std_{g}")
            nc.scalar.dma_start(out=std_t, in_=std_d[n0 : n0 + P, :])
            b0t = bpool.tile([P, O0], F32, tag="b0", name=f"b0_{g}")
            nc.scalar.dma_start(out=b0t, in_=b0_d[n0 : n0 + P, :])
            # col-space biases (small, fp32, -1 already folded in on host)
            b1t = bpool.tile([P, OC1, P], F32, tag="b1", name=f"b1_{g}")
            for oc in range(OC1):
                nc.scalar.dma_start(out=b1t[:, oc, :], in_=b1_d[g, oc])
            b2t = bpool.tile([P, P], F32, tag="b2", name=f"b2_{g}")
            nc.scalar.dma_start(out=b2t, in_=b2_d[g])
            b3t = bpool.tile([O3, P], F32, tag="b3", name=f"b3_{g}")
            nc.scalar.dma_start(out=b3t, in_=b3_d[g])
            w0ts = []
            for c0 in range(0, O0, OCH0):
                wt = w0pool.tile([P, OCH0, OBS], BF16, tag="w0", name=f"w0_{g}_{c0}")
                nc.scalar.dma_start(out=wt, in_=w0_d[n0 : n0 + P, c0 : c0 + OCH0, :])
                w0ts.append(wt)
            return obs_t, mean_t, std_t, b0t, b1t, b2t, b3t, w0ts

        def emit_row_compute(g, dmas):
            """Row-space phase: norm + L0 on DVE -> x1' = ELU(y0)+1 (bf16)."""
            obs_t, mean_t, std_t, b0t, b1t, b2t, b3t, w0ts = dmas

            # ---- obs normalization: x0 = clip((obs-mean)/std, -5, 5) ----
            nmean = spool.tile([P, OBS], F32, tag="nmean", name=f"nmean_{g}")
            nc.vector.tensor_scalar_mul(nmean, mean_t, -1.0)
            rstd = spool.tile([P, OBS], F32, tag="rstd", name=f"rstd_{g}")
            nc.vector.reciprocal(rstd, std_t)
            x0 = spool.tile([P, OBS], F32, tag="x0", name=f"x0_{g}")
            nc.vector.tensor_add(x0, obs_t, nmean)
            nc.vector.tensor_mul(x0, x0, rstd)
            x0h = xpool.tile([P, OBS], BF16, tag="x0h", name=f"x0h_{g}")
            nc.vector.tensor_scalar(
                out=x0h, in0=x0, scalar1=-5.0, scalar2=5.0,
                op0=ALU.max, op1=ALU.min,
            )

            # ---- L0: y0[t, o] = sum_i W0[t, o, i] * x0[t, i] ----
            y0 = xpool.tile([P, O0], F32, tag="y0", name=f"y0_{g}")
            for ch, c0 in enumerate(range(0, O0, OCH0)):
                wt = w0ts[ch]
                x_b = bass.AP(
                    tensor=x0h.tensor,
                    offset=x0h.offset,
                    ap=[x0h.ap[0], [0, OCH0], x0h.ap[1]],
                )
                nc.vector.tensor_mul(wt, wt, x_b)
                nc.vector.reduce_sum(
                    out=y0[:, c0 : c0 + OCH0], in_=wt, axis=mybir.AxisListType.X
                )
            nc.vector.tensor_add(y0, y0, b0t)
            # x1' = ELU(y0)+1 = exp(min(y0,0)) + max(y0,0), in bf16
            e0 = spool.tile([P, O0], F32, tag="e0", name=f"e0_{g}")
            nc.vector.tensor_scalar_min(e0, y0, 0.0)
            nc.scalar.activation(e0, e0, AF.Exp)
            x1h = xpool.tile([P, O0], BF16, tag="x1h", name=f"x1h_{g}")
            nc.vector.scalar_tensor_tensor(
                out=x1h, in0=y0, scalar=0.0, in1=e0, op0=ALU.max, op1=ALU.add,
            )

            # absorb the bias-DMA waits now so the ELU epilogue ops in the
            # col phase carry only their PSUM wait
            b1a = bpool.tile([P, OC1, P], F32, tag="b1a", name=f"b1a_{g}")
            nc.vector.tensor_scalar_mul(b1a, b1t, 1.0)
            b2a = bpool.tile([P, P], F32, tag="b2a", name=f"b2a_{g}")
            nc.vector.tensor_scalar_mul(b2a, b2t, 1.0)
            b3a = bpool.tile([O3, P], F32, tag="b3a", name=f"b3a_{g}")
            nc.vector.tensor_scalar_mul(b3a, b3t, 1.0)
            return x1h, b1a, b2a, b3a

        def elu_chunk(yps, babs, out_ap, g, li, oc):
            """out = exp(min(y,0)) + max(y,0) with y = psum + bias, bf16."""
            t0 = spool.tile([P, P], F32, tag="t0", name=f"t0_{g}_{li}_{oc}")
            nc.vector.tensor_add(t0, yps, babs)
            e = spool.tile([P, P], F32, tag="el", name=f"el_{g}_{li}_{oc}")
            nc.vector.tensor_scalar_min(e, t0, 0.0)
            nc.scalar.activation(e, e, AF.Exp)
            nc.vector.scalar_tensor_tensor(
                out=out_ap, in0=t0, scalar=0.0, in1=e, op0=ALU.max, op1=ALU.add,
            )

        def emit_x1prep(g, x1h):
            """x1' [t, 512] -> x1T [ci][i, t] (bf16) via PE transposes.
            Emitted directly after group g's row compute so the DVE copies
            don't queue behind the NEXT group's L0 work."""
            x1T = xpool.tile([P, CI1, P], BF16, tag="x1T", name=f"x1T_{g}")
            for ci in range(CI1):
                pst = ppool.tile([P, P], BF16, tag="pst", bufs=2, name=f"pst_{g}_{ci}")
                nc.tensor.transpose(pst, x1h[:, ci * P : (ci + 1) * P], ident_h)
                nc.vector.tensor_copy(x1T[:, ci, :], pst)
            return x1T

        def emit_col(g, x1T, b1a, b2a, b3a):
            """Column-space phase on PE: L1/L2/L3 as per-teacher
            LDWEIGHTS+MATMUL pairs, epilogues on DVE/ACT."""
            n0 = g * P

            # ---- L1 ----
            yps1 = [
                ppool.tile([P, P], F32, tag=f"yps1_{oc}", name=f"yps1_{g}_{oc}")
                for oc in range(OC1)
            ]
            for tb in range(0, P, TS1):
                wts = []
                for ci in range(CI1):
                    wt = wcpool.tile(
                        [P, TS1, O1], BF16, tag="wc", name=f"w1_{g}_{tb}_{ci}"
                    )
                    nc.sync.dma_start(out=wt, in_=w1_d[g, ci, :, tb : tb + TS1, :])
                    wts.append(wt)
                for tl in range(TS1):
                    t = tb + tl
                    for oc in range(OC1):
                        for ci in range(CI1):
                            nc.tensor.matmul(
                                yps1[oc][:, t : t + 1],
                                lhsT=wts[ci][:, tl, oc * P : (oc + 1) * P],
                                rhs=x1T[:, ci, t : t + 1],
                                start=(ci == 0),
                                stop=(ci == CI1 - 1),
                            )
            x2T = xpool.tile([P, CI2, P], BF16, tag="x2T", name=f"x2T_{g}")
            for oc in range(OC1):
                elu_chunk(yps1[oc], b1a[:, oc, :], x2T[:, oc, :], g, 1, oc)

            # ---- L2 ----
            yps2 = ppool.tile([P, P], F32, tag="yps2", name=f"yps2_{g}")
            for tb in range(0, P, TS2):
                wts = []
                for ci in range(CI2):
                    wt = wcpool.tile(
                        [P, TS2, O2], BF16, tag="wc", name=f"w2_{g}_{tb}_{ci}"
                    )
                    nc.sync.dma_start(out=wt, in_=w2_d[g, ci, :, tb : tb + TS2, :])
                    wts.append(wt)
                for tl in range(TS2):
                    t = tb + tl
                    for ci in range(CI2):
                        nc.tensor.matmul(
                            yps2[:, t : t + 1],
                            lhsT=wts[ci][:, tl, :],
                            rhs=x2T[:, ci, t : t + 1],
                            start=(ci == 0),
                            stop=(ci == CI2 - 1),
                        )
            x3T = xpool.tile([P, P], BF16, tag="x3T", name=f"x3T_{g}")
            elu_chunk(yps2, b2a, x3T, g, 2, 0)

            # ---- L3 ----
            w3t = w3pool.tile([P, P, O3], BF16, tag="w3", name=f"w3_{g}")
            nc.sync.dma_start(out=w3t, in_=w3_d[g])
            yps3 = ppool.tile([O3, P], F32, tag="yps3", name=f"yps3_{g}")
            for t in range(P):
                nc.tensor.matmul(
                    yps3[:, t : t + 1],
                    lhsT=w3t[:, t, :],
                    rhs=x3T[:, t : t + 1],
                    start=True,
                    stop=True,
                )
            y3 = spool.tile([O3, P], F32, tag="y3", name=f"y3_{g}")
            nc.vector.tensor_add(y3, yps3, b3a)
            nc.scalar.activation(y3, y3, AF.Tanh)
            pso = ppool.tile([P, O3], F32, tag="pso", name=f"pso_{g}")
            nc.tensor.transpose(pso, y3, ident_f)
            yt = spool.tile([P, O3], F32, tag="yt", name=f"yt_{g}")
            nc.scalar.copy(yt, pso)
            nc.scalar.dma_start(out=out_d[n0 : n0 + P, :], in_=yt)

        # Emission order (G=2): row0, prep0, row1, col0, prep1, col1.
        # - prep(g)'s DVE copies right after row(g) so they don't queue
        #   behind the next group's L0 on the Vector sequencer.
        # - prep(g+1)'s PE transposes AFTER col(g)'s matmuls so they don't
        #   head-of-line block the PE stream waiting on x1h(g+1).
        dmas = [emit_row_dmas(g) for g in range(G)]
        rows, x1Ts = [], []
        for g in range(G):
            x1h, b1a, b2a, b3a = emit_row_compute(g, dmas[g])
            rows.append((b1a, b2a, b3a))
            if g == 0:
                x1Ts.append(emit_x1prep(0, x1h))
            else:
                emit_col(g - 1, x1Ts[g - 1], *rows[g - 1])
                x1Ts.append(emit_x1prep(g, x1h))
        emit_col(G - 1, x1Ts[G - 1], *rows[G - 1])

    nc.compile()
    return nc


def _get_nc():
    if "nc" not in _cached:
        _cached["nc"] = _build_bass()
    return _cached["nc"]


def _pack_core_inputs(full, c):
    """Shard + lay out one core's inputs (bf16 weights, pretransposed)."""
    sl = slice(c * NPC, (c + 1) * NPC)
    f32 = np.float32
    w0 = np.asarray(full["W0"])[sl].astype(NPBF16)          # [NPC, 512, 48]
    w1 = np.asarray(full["W1"])[sl].astype(NPBF16)          # [NPC, 256, 512]
    w2 = np.asarray(full["W2"])[sl].astype(NPBF16)          # [NPC, 128, 256]
    w3 = np.asarray(full["W3"])[sl].astype(NPBF16)          # [NPC, 12, 128]
    # fold the ELU "-1" into the next layer's bias: b' = b - W_bf16 @ 1
    b1p = np.asarray(full["b1"])[sl] - w1.astype(f32).sum(-1)
    b2p = np.asarray(full["b2"])[sl] - w2.astype(f32).sum(-1)
    b3p = np.asarray(full["b3"])[sl] - w3.astype(f32).sum(-1)
    # W1T[g, ci, i, t, oc*128+o] = W1[g*128+t, oc*128+o, ci*128+i]
    w1t = np.ascontiguousarray(
        w1.reshape(G, P, OC1, P, CI1, P).transpose(0, 4, 5, 1, 2, 3)
        .reshape(G, CI1, P, P, O1)
    )
    b1t = np.ascontiguousarray(
        b1p.reshape(G, P, OC1, P).transpose(0, 2, 3, 1).astype(f32)
    )
    w2t = np.ascontiguousarray(
        w2.reshape(G, P, P, CI2, P).transpose(0, 3, 4, 1, 2)
    )
    b2t = np.ascontiguousarray(b2p.reshape(G, P, P).transpose(0, 2, 1).astype(f32))
    w3t = np.ascontiguousarray(w3.reshape(G, P, O3, P).transpose(0, 3, 1, 2))
    b3t = np.ascontiguousarray(b3p.reshape(G, P, O3).transpose(0, 2, 1).astype(f32))
    return {
        "obs": np.ascontiguousarray(np.asarray(full["obs"])[sl]),
        "mean": np.ascontiguousarray(np.asarray(full["mean"])[sl]),
        "std": np.ascontiguousarray(np.asarray(full["std"])[sl]),
        "W0": np.ascontiguousarray(w0),
        "b0": np.ascontiguousarray(np.asarray(full["b0"])[sl]),
        "W1T": w1t, "b1T": b1t,
        "W2T": w2t, "b2T": b2t,
        "W3T": w3t, "b3T": b3t,
    }


def kernel(obs, mean, std, W0, b0, W1, b1, W2, b2, W3, b3, _trace=False):
    nc = _get_nc()
    full = {
        "obs": obs, "mean": mean, "std": std,
        "W0": W0, "b0": b0, "W1": W1, "b1": b1,
        "W2": W2, "b2": b2, "W3": W3, "b3": b3,
    }
    in_maps = [_pack_core_inputs(full, c) for c in range(N_CORES)]
    res = run_bass_kernel_spmd(
        nc, in_maps, core_ids=list(range(N_CORES)), trace=_trace
    )
    _cached["last_results"] = res
    out = np.concatenate([res.results[c]["out"] for c in range(N_CORES)], axis=0)
    return out
